# revision 2
# baseline (speedup 1.0000x reference)
"""ODConv1d Trainium2 kernel (data-parallel over batch across 8 NeuronCores).

Reference computation (per sample b):
    pooled = mean_l x[b]                                  # [C]
    h      = relu(bn(pooled @ w_fc.T))                    # [A]
    ch_att  = sigmoid(h @ w_ch.T + b_ch)                  # [C]
    fil_att = sigmoid(h @ w_fil.T + b_fil)                # [O]
    sp_att  = sigmoid(h @ w_sp.T + b_sp)                  # [K]
    ker_att = softmax(h @ w_ker.T + b_ker)                # [KN]
    agg_w  = einsum('k,n,noik->oik', sp_att, ker_att, weight)
    out[b] = fil_att[:,None] * conv1d(ch_att[:,None] * x[b].T, agg_w, pad=1)

Key restructuring: all three elementwise attentions fold into the aggregated
weight  W'[o,i,k] = fil[o]*ch[i]*sp[k] * sum_n ker[n] * weight[n,o,i,k],
so the big tensors (x, out) only flow through transpose + matmul.
"""

import numpy as np
import ml_dtypes

B, L_FULL, C, O, KK, KN, A = 32, 4096, 256, 256, 3, 4, 16
BN_EPS = 1e-5
P = 128
N_CORES = 8
NB = B // N_CORES  # samples per core

# params column layout (f32 [128, PCOLS])
_PC_FCT = 0         # [:, 0:32]    w_fcT  [p, ci*16+a]
_PC_CHT = 32        # [0:16, 32:288]  w_chT [a, c]
_PC_CAT = 288       # [0:16, 288:551] [w_fil.T | w_sp.T | w_ker.T]
_PC_BNS = 551       # [0:16, 551] bn scale (incl /L)
_PC_BNB = 552       # [0:16, 552] bn bias
_PC_BCH = 553       # [:, 553:555] b_chT
_PC_BCAT = 555      # [0, 555:818] [b_fil | b_sp | b_ker]
_PC_ONES = 818      # [0, 818:946] ones (f32 broadcast lhsT)
PCOLS = 946

_BUILD_CACHE = {}


def _build(n_samples, L):
    """Build the Bass module for one core processing `n_samples` of length L."""
    from contextlib import ExitStack
    import concourse.bass as bass  # noqa: F401
    import concourse.mybir as mybir
    import concourse.tile as tile
    from concourse import bacc

    dt = mybir.dt
    F32 = dt.float32
    BF16 = dt.bfloat16
    AF = mybir.ActivationFunctionType
    OP = mybir.AluOpType
    AX = mybir.AxisListType

    NL = L // 128      # number of 128-row l-tiles per sample
    NJ = NL // 4       # transpose copy groups (4 tiles of 128 -> 512 cols)
    NG = L // 512      # conv l-groups

    nc = bacc.Bacc(None, target_bir_lowering=False)
    names = {}

    with tile.TileContext(nc) as tc, ExitStack() as ctx:
        dram = ctx.enter_context(tc.tile_pool(name="dram", bufs=1, space="DRAM"))
        x_in = dram.tile([n_samples, L, C], F32, kind="ExternalInput")
        bank_d = dram.tile([P, KN, 2, KK, O], BF16, kind="ExternalInput")
        par_d = dram.tile([P, PCOLS], F32, kind="ExternalInput")
        con_d = dram.tile([P, 256], BF16, kind="ExternalInput")
        out_d = dram.tile([n_samples, O, L], F32, kind="ExternalOutput")
        names["x"] = x_in.name
        names["bank"] = bank_d.name
        names["params"] = par_d.name
        names["consts"] = con_d.name
        names["out"] = out_d.name

        cpool = ctx.enter_context(tc.tile_pool(name="const", bufs=1))
        xnp = ctx.enter_context(tc.tile_pool(name="xnat", bufs=2))
        xtp = ctx.enter_context(tc.tile_pool(name="xt", bufs=2))
        tpsum = ctx.enter_context(tc.tile_pool(name="tpsum", bufs=2, space="PSUM"))
        cpsum = ctx.enter_context(tc.tile_pool(name="cpsum", bufs=4, space="PSUM"))
        hpsum = ctx.enter_context(tc.tile_pool(name="hpsum", bufs=1, space="PSUM"))
        hsb = ctx.enter_context(tc.tile_pool(name="hsb", bufs=2))
        accp = ctx.enter_context(tc.tile_pool(name="accp", bufs=2))
        aggp = ctx.enter_context(tc.tile_pool(name="aggp", bufs=2))
        outp = ctx.enter_context(tc.tile_pool(name="outp", bufs=3))

        # constants
        bank = cpool.tile([P, KN, 2, KK, O], BF16)
        nc.sync.dma_start(out=bank[:], in_=bank_d[:])
        par = cpool.tile([P, PCOLS], F32)
        nc.sync.dma_start(out=par[:], in_=par_d[:])
        con = cpool.tile([P, 256], BF16)
        nc.sync.dma_start(out=con[:], in_=con_d[:])

        ident = con[:, 0:128]
        ones_bf = con[0:1, 128:256]
        ones_f32 = par[0:1, _PC_ONES:_PC_ONES + 128]
        bn_scale = par[0:16, _PC_BNS:_PC_BNS + 1]
        bn_bias = par[0:16, _PC_BNB:_PC_BNB + 1]
        catT = par[0:16, _PC_CAT:_PC_CAT + O + KK + KN]
        b_cat = par[0:1, _PC_BCAT:_PC_BCAT + O + KK + KN]

        for b in range(n_samples):
            # ---- load x[b] natural layout, cast f32 -> bf16 during DMA ----
            xn = xnp.tile([P, NL, C], BF16, tag="xn")
            nc.gpsimd.dma_start(
                out=xn[:], in_=x_in[b].rearrange("(n p) c -> p n c", p=P)
            )

            # ---- transpose to [c_part, l] + pooled row-sums via accum ----
            xt = xtp.tile([P, 2, L + 2], BF16, tag="xt")
            nc.vector.memset(xt[:, :, 0:1], 0.0)
            nc.vector.memset(xt[:, :, L + 1:L + 2], 0.0)
            pp = hsb.tile([P, 2, NJ], F32, tag="pp")
            for ci in range(2):
                for j in range(NJ):
                    tp = tpsum.tile([P, 512], BF16, tag="tp")
                    for q in range(4):
                        n = j * 4 + q
                        nc.tensor.transpose(
                            tp[:, q * 128:(q + 1) * 128],
                            xn[:, n, ci * 128:(ci + 1) * 128],
                            ident,
                        )
                    nc.scalar.activation(
                        xt[:, ci, 1 + j * 512:1 + (j + 1) * 512],
                        tp[:],
                        AF.Copy,
                        accum_out=pp[:, ci, j:j + 1],
                    )
            pooledT = hsb.tile([P, 2], F32, tag="pooledT")
            nc.vector.tensor_reduce(pooledT[:], pp[:], AX.X, op=OP.add)

            # ---- attention head (all tiny) ----
            h_ps = hpsum.tile([16, 1], F32, tag="h")
            for ci in range(2):
                nc.tensor.matmul(
                    h_ps[:],
                    par[:, ci * 16:(ci + 1) * 16],
                    pooledT[:, ci:ci + 1],
                    start=(ci == 0),
                    stop=(ci == 1),
                )
            h_sb = hsb.tile([16, 1], F32, tag="h_sb")
            nc.scalar.activation(h_sb[:], h_ps[:], AF.Relu, bias=bn_bias, scale=bn_scale)

            z_ps = hpsum.tile([1, O + KK + KN], F32, tag="h")
            nc.tensor.matmul(z_ps[:], h_sb[:], catT, start=True, stop=True)
            zb = hsb.tile([1, O + KK + KN], F32, tag="zb")
            nc.vector.tensor_add(zb[:], z_ps[:], b_cat)
            att = hsb.tile([1, O + KK + KN], F32, tag="att")
            nc.scalar.activation(att[0:1, 0:O + KK], zb[0:1, 0:O + KK], AF.Sigmoid)
            kmax = hsb.tile([1, 2], F32, tag="kmax")
            nc.vector.tensor_reduce(
                kmax[0:1, 0:1], zb[0:1, O + KK:O + KK + KN], AX.X, op=OP.max
            )
            nc.vector.tensor_scalar_mul(kmax[0:1, 1:2], kmax[0:1, 0:1], -1.0)
            ksum = hsb.tile([1, 2], F32, tag="ksum")
            nc.scalar.activation(
                att[0:1, O + KK:O + KK + KN],
                zb[0:1, O + KK:O + KK + KN],
                AF.Exp,
                bias=kmax[0:1, 1:2],
                accum_out=ksum[0:1, 0:1],
            )
            nc.vector.reciprocal(ksum[0:1, 1:2], ksum[0:1, 0:1])

            chz_ps = hpsum.tile([P, 2], F32, tag="h")
            for ci in range(2):
                nc.tensor.matmul(
                    chz_ps[:, ci:ci + 1],
                    par[0:16, _PC_CHT + ci * 128:_PC_CHT + (ci + 1) * 128],
                    h_sb[:],
                    start=True,
                    stop=True,
                )
            ch_sb = hsb.tile([P, 2], F32, tag="ch_sb")
            for ci in range(2):
                nc.scalar.activation(
                    ch_sb[:, ci:ci + 1],
                    chz_ps[:, ci:ci + 1],
                    AF.Sigmoid,
                    bias=par[:, _PC_BCH + ci:_PC_BCH + ci + 1],
                )

            # fo[k, o] = sp_att[k] * fil_att[o] / ksum   (bf16)
            fo = hsb.tile([1, KK, O], BF16, tag="fo")
            for k in range(KK):
                nc.vector.tensor_scalar(
                    fo[0:1, k],
                    att[0:1, 0:O],
                    att[0:1, O + k:O + k + 1],
                    ksum[0:1, 1:2],
                    op0=OP.mult,
                    op1=OP.mult,
                )
            fo_ps = hpsum.tile([P, KK * O], F32, tag="h")
            fo_flat = fo.rearrange("p a b -> p (a b)")
            nc.tensor.matmul(
                fo_ps[:, 0:512], ones_bf, fo_flat[0:1, 0:512], start=True, stop=True
            )
            nc.tensor.matmul(
                fo_ps[:, 512:KK * O], ones_bf, fo_flat[0:1, 512:KK * O],
                start=True, stop=True,
            )
            fo_bc = hsb.tile([P, KK, O], BF16, tag="fo_bc")
            nc.vector.tensor_copy(fo_bc.rearrange("p a b -> p (a b)"), fo_ps[:])

            e_ps = hpsum.tile([P, KN], F32, tag="h")
            nc.tensor.matmul(
                e_ps[:], ones_f32, att[0:1, O + KK:O + KK + KN], start=True, stop=True
            )
            kerb = hsb.tile([P, KN], F32, tag="kerb")
            nc.vector.tensor_copy(kerb[:], e_ps[:])

            # ---- aggregate dynamic weight  (bf16, DVE) ----
            a0 = accp.tile([P, 2, KK, O], BF16, tag="a0")
            a1 = accp.tile([P, 2, KK, O], BF16, tag="a1")
            nc.vector.tensor_scalar_mul(a0[:], bank[:, 0], kerb[:, 0:1])
            nc.vector.scalar_tensor_tensor(
                a1[:], bank[:, 1], kerb[:, 1:2], a0[:], op0=OP.mult, op1=OP.add
            )
            nc.vector.scalar_tensor_tensor(
                a0[:], bank[:, 2], kerb[:, 2:3], a1[:], op0=OP.mult, op1=OP.add
            )
            nc.vector.scalar_tensor_tensor(
                a1[:], bank[:, 3], kerb[:, 3:4], a0[:], op0=OP.mult, op1=OP.add
            )
            aggT = aggp.tile([P, 2, KK, O], BF16, tag="aggT")
            for ci in range(2):
                nc.vector.scalar_tensor_tensor(
                    aggT[:, ci],
                    a1[:, ci],
                    ch_sb[:, ci:ci + 1],
                    fo_bc[:],
                    op0=OP.mult,
                    op1=OP.mult,
                )

            # ---- convolution: out[o, l] accumulated over (ci, k) ----
            for m in range(2):
                osb = outp.tile([P, L], F32, tag="osb")
                for lg in range(NG):
                    cp = cpsum.tile([P, 512], F32, tag="cp")
                    idx = 0
                    for ci in range(2):
                        for k in range(KK):
                            nc.tensor.matmul(
                                cp[:],
                                aggT[:, ci, k, m * 128:(m + 1) * 128],
                                xt[:, ci, lg * 512 + k:lg * 512 + k + 512],
                                start=(idx == 0),
                                stop=(idx == 5),
                            )
                            idx += 1
                    nc.vector.tensor_copy(osb[:, lg * 512:(lg + 1) * 512], cp[:])
                nc.sync.dma_start(
                    out=out_d[b, m * 128:(m + 1) * 128, :], in_=osb[:]
                )

    nc.compile()
    return nc, names


def _host_prep(weight, w_fc, bn_gamma, bn_beta, bn_mean, bn_var,
               w_ch, b_ch, w_fil, b_fil, w_sp, b_sp, w_ker, b_ker, L):
    bf16 = ml_dtypes.bfloat16
    bank = np.ascontiguousarray(
        weight.reshape(KN, O, 2, P, KK).transpose(3, 0, 2, 4, 1)
    ).astype(bf16)  # [p, n, ci, k, o]

    par = np.zeros((P, PCOLS), np.float32)
    # w_fcT [p, ci*16+a]
    par[:, 0:32] = w_fc.T.reshape(2, P, A).transpose(1, 0, 2).reshape(P, 32)
    par[0:A, _PC_CHT:_PC_CHT + C] = w_ch.T
    par[0:A, _PC_CAT:_PC_CAT + O] = w_fil.T
    par[0:A, _PC_CAT + O:_PC_CAT + O + KK] = w_sp.T
    par[0:A, _PC_CAT + O + KK:_PC_CAT + O + KK + KN] = w_ker.T
    r = bn_gamma / np.sqrt(bn_var + BN_EPS)
    par[0:A, _PC_BNS] = r / L
    par[0:A, _PC_BNB] = bn_beta - bn_mean * r
    par[:, _PC_BCH:_PC_BCH + 2] = b_ch.reshape(2, P).T
    par[0, _PC_BCAT:_PC_BCAT + O] = b_fil
    par[0, _PC_BCAT + O:_PC_BCAT + O + KK] = b_sp
    par[0, _PC_BCAT + O + KK:_PC_BCAT + O + KK + KN] = b_ker
    par[0, _PC_ONES:_PC_ONES + 128] = 1.0

    con = np.zeros((P, 256), bf16)
    con[:, 0:128] = np.eye(P, dtype=bf16)
    con[0, 128:256] = bf16(1.0)
    return bank, par, con


def run_cores(x, weight, w_fc, bn_gamma, bn_beta, bn_mean, bn_var,
              w_ch, b_ch, w_fil, b_fil, w_sp, b_sp, w_ker, b_ker,
              n_cores=N_CORES, trace=False):
    """Shard x over cores, run, gather. x: [n_cores*ns, L, C] float32."""
    from concourse.bass_utils import run_bass_kernel_spmd

    x = np.ascontiguousarray(np.asarray(x, np.float32))
    ntot, L, _ = x.shape
    assert ntot % n_cores == 0
    ns = ntot // n_cores

    key = (ns, L)
    if key not in _BUILD_CACHE:
        _BUILD_CACHE[key] = _build(ns, L)
    nc, names = _BUILD_CACHE[key]

    bank, par, con = _host_prep(
        np.asarray(weight, np.float32), np.asarray(w_fc, np.float32),
        np.asarray(bn_gamma, np.float32), np.asarray(bn_beta, np.float32),
        np.asarray(bn_mean, np.float32), np.asarray(bn_var, np.float32),
        np.asarray(w_ch, np.float32), np.asarray(b_ch, np.float32),
        np.asarray(w_fil, np.float32), np.asarray(b_fil, np.float32),
        np.asarray(w_sp, np.float32), np.asarray(b_sp, np.float32),
        np.asarray(w_ker, np.float32), np.asarray(b_ker, np.float32), L)

    in_maps = []
    for c in range(n_cores):
        in_maps.append({
            names["x"]: np.ascontiguousarray(x[c * ns:(c + 1) * ns]),
            names["bank"]: bank,
            names["params"]: par,
            names["consts"]: con,
        })
    res = run_bass_kernel_spmd(nc, in_maps, core_ids=list(range(n_cores)),
                               trace=trace)
    out = np.concatenate([r[names["out"]] for r in res.results], axis=0)
    return out, res


def kernel(**inputs):
    out, _ = run_cores(**inputs)
    return out


# revision 7
# speedup vs baseline: 1.0998x; 1.0998x over previous
"""ODConv1d Trainium2 kernel (data-parallel over batch across 8 NeuronCores).

Reference computation (per sample b):
    pooled = mean_l x[b]                                  # [C]
    h      = relu(bn(pooled @ w_fc.T))                    # [A]
    ch_att  = sigmoid(h @ w_ch.T + b_ch)                  # [C]
    fil_att = sigmoid(h @ w_fil.T + b_fil)                # [O]
    sp_att  = sigmoid(h @ w_sp.T + b_sp)                  # [K]
    ker_att = softmax(h @ w_ker.T + b_ker)                # [KN]
    agg_w  = einsum('k,n,noik->oik', sp_att, ker_att, weight)
    out[b] = fil_att[:,None] * conv1d(ch_att[:,None] * x[b].T, agg_w, pad=1)

Key restructurings:
  * all three elementwise attentions fold into the aggregated weight
    W'[o,i,k] = fil[o]*ch[i]*sp[k] * sum_n ker[n] * weight[n,o,i,k],
    so the big tensors (x, out) only flow through transpose + matmul;
  * software pipelining across samples: sample b+1's x-load, PE transposes
    and attention head are interleaved into sample b's conv matmuls so the
    PE never idles (keeps the HAM clock-gate at full speed);
  * pooling rides the transpose PSUM->SBUF copies via accum_out.
"""

import numpy as np
import ml_dtypes

B, L_FULL, C, O, KK, KN, A = 32, 4096, 256, 256, 3, 4, 16
BN_EPS = 1e-5
P = 128
N_CORES = 8
NB = B // N_CORES  # samples per core

# params column layout (f32 [128, PCOLS])
_PC_FCT = 0         # [:, 0:32]    w_fcT  [p, ci*16+a]
_PC_CHT = 32        # [0:16, 32:288]  w_chT [a, c]
_PC_CAT = 288       # [0:16, 288:551] [w_fil.T | w_sp.T | w_ker.T]
_PC_BNS = 551       # [0:16, 551] bn scale (incl /L)
_PC_BNB = 552       # [0:16, 552] bn bias
_PC_BCH = 553       # [:, 553:555] b_chT
_PC_BCAT = 555      # [0, 555:818] [b_fil | b_sp | b_ker]
_PC_ONES = 818      # [0, 818:946] ones (f32 broadcast lhsT)
PCOLS = 946

_BUILD_CACHE = {}


def _build(n_samples, L):
    """Build the Bass module for one core processing `n_samples` of length L."""
    from contextlib import ExitStack
    import concourse.bass as bass  # noqa: F401
    import concourse.mybir as mybir
    import concourse.tile as tile
    from concourse import bacc

    dt = mybir.dt
    F32 = dt.float32
    BF16 = dt.bfloat16
    AF = mybir.ActivationFunctionType
    OP = mybir.AluOpType
    AX = mybir.AxisListType

    NL = L // 128      # 128-row l-tiles per sample
    NJ = NL // 4       # transpose groups (4 tiles of 128 -> one 512-col copy)
    NG = L // 512      # conv l-groups
    NCH = 4            # x-load chunks per sample
    NLC = NL // NCH    # l-tiles per chunk
    assert NL % NCH == 0 and NL % 4 == 0 and NG >= 2

    nc = bacc.Bacc(None, target_bir_lowering=False)
    names = {}

    with tile.TileContext(nc) as tc, ExitStack() as ctx:
        dram = ctx.enter_context(tc.tile_pool(name="dram", bufs=1, space="DRAM"))
        x_in = dram.tile([n_samples, L, C], F32, kind="ExternalInput")
        bank_d = dram.tile([P, KN, 2, KK, O], BF16, kind="ExternalInput")
        par_d = dram.tile([P, PCOLS], F32, kind="ExternalInput")
        con_d = dram.tile([P, 256], BF16, kind="ExternalInput")
        out_d = dram.tile([n_samples, O, L], F32, kind="ExternalOutput")
        names["x"] = x_in.name
        names["bank"] = bank_d.name
        names["params"] = par_d.name
        names["consts"] = con_d.name
        names["out"] = out_d.name

        cpool = ctx.enter_context(tc.tile_pool(name="const", bufs=1))
        xfp = ctx.enter_context(tc.tile_pool(name="xf", bufs=2))
        xnp = ctx.enter_context(tc.tile_pool(name="xnat", bufs=2 * NCH))
        xtp = ctx.enter_context(tc.tile_pool(name="xt", bufs=2))
        tpsum = ctx.enter_context(tc.tile_pool(name="tpsum", bufs=3, space="PSUM"))
        cpsum = ctx.enter_context(tc.tile_pool(name="cpsum", bufs=4, space="PSUM"))
        hpsum = ctx.enter_context(tc.tile_pool(name="hpsum", bufs=1, space="PSUM"))
        hsb = ctx.enter_context(tc.tile_pool(name="hsb", bufs=2))
        accp = ctx.enter_context(tc.tile_pool(name="accp", bufs=2))
        aggp = ctx.enter_context(tc.tile_pool(name="aggp", bufs=2))
        outp = ctx.enter_context(tc.tile_pool(name="outp", bufs=3))

        # constants: consts first (identity unblocks transposes), bank last
        con = cpool.tile([P, 256], BF16)
        nc.sync.dma_start(out=con[:], in_=con_d[:])
        par = cpool.tile([P, PCOLS], F32)
        nc.sync.dma_start(out=par[:], in_=par_d[:])

        ident = con[:, 0:128]
        ones_bf = con[0:1, 128:256]
        ones_f32 = par[0:1, _PC_ONES:_PC_ONES + 128]
        bn_scale = par[0:16, _PC_BNS:_PC_BNS + 1]
        bn_bias = par[0:16, _PC_BNB:_PC_BNB + 1]
        catT = par[0:16, _PC_CAT:_PC_CAT + O + KK + KN]
        b_cat = par[0:1, _PC_BCAT:_PC_BCAT + O + KK + KN]

        xn_chunks = {}   # b -> list of NCH bf16 tiles [P, NLC, C]
        xt_tiles = {}
        pp_tiles = {}
        agg_tiles = {}

        def load(b):
            """Issue x[b] chunk loads. Sample 0 goes HWDGE f32 + DVE cast
            (fast start, separate queue); the rest use SWDGE casting DMA."""
            xv = x_in[b].rearrange("(n p) c -> p n c", p=P)
            tiles = []
            for ch in range(NCH):
                sl = slice(ch * NLC, (ch + 1) * NLC)
                xnt = xnp.tile([P, NLC, C], BF16, tag="xn")
                if b == 0:
                    xf = xfp.tile([P, NLC, C], F32, tag="xf")
                    nc.sync.dma_start(out=xf[:], in_=xv[:, sl, :])
                    nc.vector.tensor_copy(xnt[:], xf[:])
                else:
                    nc.gpsimd.dma_start(out=xnt[:], in_=xv[:, sl, :])
                tiles.append(xnt)
            xn_chunks[b] = tiles

        def tp_group(b, j):
            """Transpose 4 l-tiles (512 cols) of sample b + copy to xt with
            pooled row-sum accumulation. Alternates ACT/DVE for the copy."""
            xt = xt_tiles[b]
            pp = pp_tiles[b]
            ci, jj = divmod(j, NJ)
            tp = tpsum.tile([P, 512], BF16, tag="tp")
            for q in range(4):
                idx = jj * 4 + q
                chunk = xn_chunks[b][idx // NLC]
                nc.tensor.transpose(
                    tp[:, q * 128:(q + 1) * 128],
                    chunk[:, idx % NLC, ci * 128:(ci + 1) * 128],
                    ident,
                )
            dst = xt[:, ci, 1 + jj * 512:1 + (jj + 1) * 512]
            acc = pp[:, ci, jj:jj + 1]
            if j % 2 == 0:
                nc.scalar.activation(dst, tp[:], AF.Copy, accum_out=acc)
            else:
                nc.vector.tensor_scalar(dst, tp[:], 0.0, 0.0, op0=OP.add,
                                        op1=OP.add, accum_out=acc)

        def tp_begin(b):
            xt = xtp.tile([P, 2, L + 2], BF16, tag="xt")
            nc.vector.memset(xt[:, :, 0:1], 0.0)
            nc.vector.memset(xt[:, :, L + 1:L + 2], 0.0)
            xt_tiles[b] = xt
            pp_tiles[b] = hsb.tile([P, 2, NJ], F32, tag="pp", name="pp")

        def head(b):
            """Attention head + dynamic-weight aggregation for sample b."""
            pp = pp_tiles[b]
            pooledT = hsb.tile([P, 2], F32, tag="pooledT")
            nc.vector.tensor_reduce(pooledT[:], pp[:], AX.X, op=OP.add)

            h_ps = hpsum.tile([16, 1], F32, tag="h")
            for ci in range(2):
                nc.tensor.matmul(
                    h_ps[:],
                    par[:, ci * 16:(ci + 1) * 16],
                    pooledT[:, ci:ci + 1],
                    start=(ci == 0),
                    stop=(ci == 1),
                )
            h_sb = hsb.tile([16, 1], F32, tag="h_sb")
            nc.scalar.activation(h_sb[:], h_ps[:], AF.Relu,
                                 bias=bn_bias, scale=bn_scale)

            z_ps = hpsum.tile([1, O + KK + KN], F32, tag="h")
            nc.tensor.matmul(z_ps[:], h_sb[:], catT, start=True, stop=True)
            zb = hsb.tile([1, O + KK + KN], F32, tag="zb")
            nc.vector.tensor_add(zb[:], z_ps[:], b_cat)
            att = hsb.tile([1, O + KK + KN], F32, tag="att")
            nc.scalar.activation(att[0:1, 0:O + KK], zb[0:1, 0:O + KK], AF.Sigmoid)
            kmax = hsb.tile([1, 2], F32, tag="kmax")
            nc.vector.tensor_reduce(
                kmax[0:1, 0:1], zb[0:1, O + KK:O + KK + KN], AX.X, op=OP.max
            )
            nc.vector.tensor_scalar_mul(kmax[0:1, 1:2], kmax[0:1, 0:1], -1.0)
            ksum = hsb.tile([1, 2], F32, tag="ksum")
            nc.scalar.activation(
                att[0:1, O + KK:O + KK + KN],
                zb[0:1, O + KK:O + KK + KN],
                AF.Exp,
                bias=kmax[0:1, 1:2],
                accum_out=ksum[0:1, 0:1],
            )
            nc.vector.reciprocal(ksum[0:1, 1:2], ksum[0:1, 0:1])

            chz_ps = hpsum.tile([P, 2], F32, tag="h")
            for ci in range(2):
                nc.tensor.matmul(
                    chz_ps[:, ci:ci + 1],
                    par[0:16, _PC_CHT + ci * 128:_PC_CHT + (ci + 1) * 128],
                    h_sb[:],
                    start=True,
                    stop=True,
                )
            ch_sb = hsb.tile([P, 2], F32, tag="ch_sb")
            for ci in range(2):
                nc.scalar.activation(
                    ch_sb[:, ci:ci + 1],
                    chz_ps[:, ci:ci + 1],
                    AF.Sigmoid,
                    bias=par[:, _PC_BCH + ci:_PC_BCH + ci + 1],
                )

            # fo[k, o] = sp_att[k] * fil_att[o] / ksum, broadcast to 128 parts
            fo = hsb.tile([1, KK, O], BF16, tag="fo")
            for k in range(KK):
                nc.vector.tensor_scalar(
                    fo[0:1, k],
                    att[0:1, 0:O],
                    att[0:1, O + k:O + k + 1],
                    ksum[0:1, 1:2],
                    op0=OP.mult,
                    op1=OP.mult,
                )
            fo_bc = hsb.tile([P, KK, O], BF16, tag="fo_bc")
            fo_flat = fo.rearrange("p a b -> p (a b)")
            fobc_flat = fo_bc.rearrange("p a b -> p (a b)")
            for s0, s1 in ((0, 512), (512, KK * O)):
                fo_ps = hpsum.tile([P, 512], F32, tag="h")
                nc.tensor.matmul(fo_ps[:, 0:s1 - s0], ones_bf,
                                 fo_flat[0:1, s0:s1], start=True, stop=True)
                nc.vector.tensor_copy(fobc_flat[:, s0:s1], fo_ps[:, 0:s1 - s0])

            e_ps = hpsum.tile([P, KN], F32, tag="h")
            nc.tensor.matmul(e_ps[:], ones_f32, att[0:1, O + KK:O + KK + KN],
                             start=True, stop=True)
            kerb = hsb.tile([P, KN], F32, tag="kerb")
            nc.vector.tensor_copy(kerb[:], e_ps[:])

            # aggregate dynamic weight (bf16, DVE)
            a0 = accp.tile([P, 2, KK, O], BF16, tag="a0")
            a1 = accp.tile([P, 2, KK, O], BF16, tag="a1")
            nc.vector.tensor_scalar_mul(a0[:], bank[:, 0], kerb[:, 0:1])
            nc.vector.scalar_tensor_tensor(
                a1[:], bank[:, 1], kerb[:, 1:2], a0[:], op0=OP.mult, op1=OP.add
            )
            nc.vector.scalar_tensor_tensor(
                a0[:], bank[:, 2], kerb[:, 2:3], a1[:], op0=OP.mult, op1=OP.add
            )
            nc.vector.scalar_tensor_tensor(
                a1[:], bank[:, 3], kerb[:, 3:4], a0[:], op0=OP.mult, op1=OP.add
            )
            aggT = aggp.tile([P, 2, KK, O], BF16, tag="aggT")
            for ci in range(2):
                nc.vector.scalar_tensor_tensor(
                    aggT[:, ci],
                    a1[:, ci],
                    ch_sb[:, ci:ci + 1],
                    fo_bc[:],
                    op0=OP.mult,
                    op1=OP.mult,
                )
            agg_tiles[b] = aggT

        # ---------------- pipelined schedule ----------------
        load(0)
        if n_samples > 1:
            load(1)
        # bank load after x[0] is queued (HWDGE; needed only for agg at ~20us)
        bank = cpool.tile([P, KN, 2, KK, O], BF16)
        nc.sync.dma_start(out=bank[:], in_=bank_d[:])

        tp_begin(0)
        for j in range(2 * NJ):
            tp_group(0, j)
        head(0)

        for b in range(n_samples):
            if b + 2 < n_samples:
                load(b + 2)
            if b + 1 < n_samples:
                tp_begin(b + 1)
            aggT = agg_tiles[b]
            xt = xt_tiles[b]
            g = 0
            for m in range(2):
                osb = outp.tile([P, L], F32, tag="osb")
                for lg in range(NG):
                    cp = cpsum.tile([P, 512], F32, tag="cp")
                    idx = 0
                    for ci in range(2):
                        for k in range(KK):
                            nc.tensor.matmul(
                                cp[:],
                                aggT[:, ci, k, m * 128:(m + 1) * 128],
                                xt[:, ci, lg * 512 + k:lg * 512 + k + 512],
                                start=(idx == 0),
                                stop=(idx == 5),
                            )
                            idx += 1
                    dst = osb[:, lg * 512:(lg + 1) * 512]
                    if lg % 2 == 0:
                        nc.vector.tensor_copy(dst, cp[:])
                    else:
                        nc.scalar.activation(dst, cp[:], AF.Copy)
                    # interleave next sample's transposes / head into the conv
                    if b + 1 < n_samples:
                        if g < NG:
                            tp_group(b + 1, 2 * g)
                            tp_group(b + 1, 2 * g + 1)
                        elif g == NG:
                            head(b + 1)
                    if lg % (NG // 2) == (NG // 2) - 1:
                        h0 = (lg // (NG // 2)) * (L // 2)
                        nc.sync.dma_start(
                            out=out_d[b, m * 128:(m + 1) * 128, h0:h0 + L // 2],
                            in_=osb[:, h0:h0 + L // 2],
                        )
                    g += 1

    nc.compile()
    return nc, names


def _host_prep(weight, w_fc, bn_gamma, bn_beta, bn_mean, bn_var,
               w_ch, b_ch, w_fil, b_fil, w_sp, b_sp, w_ker, b_ker, L):
    bf16 = ml_dtypes.bfloat16
    bank = np.ascontiguousarray(
        weight.reshape(KN, O, 2, P, KK).transpose(3, 0, 2, 4, 1)
    ).astype(bf16)  # [p, n, ci, k, o]

    par = np.zeros((P, PCOLS), np.float32)
    # w_fcT [p, ci*16+a]
    par[:, 0:32] = w_fc.T.reshape(2, P, A).transpose(1, 0, 2).reshape(P, 32)
    par[0:A, _PC_CHT:_PC_CHT + C] = w_ch.T
    par[0:A, _PC_CAT:_PC_CAT + O] = w_fil.T
    par[0:A, _PC_CAT + O:_PC_CAT + O + KK] = w_sp.T
    par[0:A, _PC_CAT + O + KK:_PC_CAT + O + KK + KN] = w_ker.T
    r = bn_gamma / np.sqrt(bn_var + BN_EPS)
    par[0:A, _PC_BNS] = r / L
    par[0:A, _PC_BNB] = bn_beta - bn_mean * r
    par[:, _PC_BCH:_PC_BCH + 2] = b_ch.reshape(2, P).T
    par[0, _PC_BCAT:_PC_BCAT + O] = b_fil
    par[0, _PC_BCAT + O:_PC_BCAT + O + KK] = b_sp
    par[0, _PC_BCAT + O + KK:_PC_BCAT + O + KK + KN] = b_ker
    par[0, _PC_ONES:_PC_ONES + 128] = 1.0

    con = np.zeros((P, 256), bf16)
    con[:, 0:128] = np.eye(P, dtype=bf16)
    con[0, 128:256] = bf16(1.0)
    return bank, par, con


def run_cores(x, weight, w_fc, bn_gamma, bn_beta, bn_mean, bn_var,
              w_ch, b_ch, w_fil, b_fil, w_sp, b_sp, w_ker, b_ker,
              n_cores=N_CORES, trace=False):
    """Shard x over cores, run, gather. x: [n_cores*ns, L, C] float32."""
    from concourse.bass_utils import run_bass_kernel_spmd

    x = np.ascontiguousarray(np.asarray(x, np.float32))
    ntot, L, _ = x.shape
    assert ntot % n_cores == 0
    ns = ntot // n_cores

    key = (ns, L)
    if key not in _BUILD_CACHE:
        _BUILD_CACHE[key] = _build(ns, L)
    nc, names = _BUILD_CACHE[key]

    bank, par, con = _host_prep(
        np.asarray(weight, np.float32), np.asarray(w_fc, np.float32),
        np.asarray(bn_gamma, np.float32), np.asarray(bn_beta, np.float32),
        np.asarray(bn_mean, np.float32), np.asarray(bn_var, np.float32),
        np.asarray(w_ch, np.float32), np.asarray(b_ch, np.float32),
        np.asarray(w_fil, np.float32), np.asarray(b_fil, np.float32),
        np.asarray(w_sp, np.float32), np.asarray(b_sp, np.float32),
        np.asarray(w_ker, np.float32), np.asarray(b_ker, np.float32), L)

    in_maps = []
    for c in range(n_cores):
        in_maps.append({
            names["x"]: np.ascontiguousarray(x[c * ns:(c + 1) * ns]),
            names["bank"]: bank,
            names["params"]: par,
            names["consts"]: con,
        })
    res = run_bass_kernel_spmd(nc, in_maps, core_ids=list(range(n_cores)),
                               trace=trace)
    out = np.concatenate([r[names["out"]] for r in res.results], axis=0)
    return out, res


def kernel(**inputs):
    out, _ = run_cores(**inputs)
    return out


# revision 10
# speedup vs baseline: 1.1449x; 1.0411x over previous
"""ODConv1d Trainium2 kernel (data-parallel over batch across 8 NeuronCores).

Reference computation (per sample b):
    pooled = mean_l x[b]                                  # [C]
    h      = relu(bn(pooled @ w_fc.T))                    # [A]
    ch_att  = sigmoid(h @ w_ch.T + b_ch)                  # [C]
    fil_att = sigmoid(h @ w_fil.T + b_fil)                # [O]
    sp_att  = sigmoid(h @ w_sp.T + b_sp)                  # [K]
    ker_att = softmax(h @ w_ker.T + b_ker)                # [KN]
    agg_w  = einsum('k,n,noik->oik', sp_att, ker_att, weight)
    out[b] = fil_att[:,None] * conv1d(ch_att[:,None] * x[b].T, agg_w, pad=1)

Key restructurings:
  * all three elementwise attentions fold into the aggregated weight
    W'[o,i,k] = fil[o]*ch[i]*sp[k] * sum_n ker[n] * weight[n,o,i,k],
    so the big tensors (x, out) only flow through transpose + matmul;
  * x is loaded with 32 consecutive rows per SBUF partition (full-bandwidth
    contiguous DMA); the PE-transpose column scramble this causes is undone
    for free by strided PSUM->SBUF copies;
  * software pipelining across samples: sample b+1's x-load, PE transposes
    and attention head are interleaved into sample b's conv matmuls so the
    PE never idles (keeps the HAM clock-gate at full speed);
  * pooling rides the transpose PSUM->SBUF copies via accum_out;
  * head biases are folded into the matmuls via a 17th "constant 1" row;
  * the dynamic-weight aggregation is split into o-halves so the next
    sample's first conv matmuls can start after half the aggregation.
"""

import numpy as np
import ml_dtypes

B, L_FULL, C, O, KK, KN, A = 32, 4096, 256, 256, 3, 4, 16
BN_EPS = 1e-5
P = 128
N_CORES = 8
NB = B // N_CORES  # samples per core

# params column layout (f32 [128, PCOLS])
_PC_FCT = 0         # [:, 0:32]      w_fcT [p, ci*16+a]
_PC_CHT = 32        # [0:17, 32:288]  rows 0:16 w_ch.T, row 16 b_ch
_PC_CAT = 288       # [0:17, 288:551] rows 0:16 [w_fil|w_sp|w_ker].T, row 16 bias
_PC_BNS = 551       # [0:16, 551] bn scale (incl /L)
_PC_BNB = 552       # [0:16, 552] bn bias
_PC_ONES = 553      # [0, 553:681] ones (f32 broadcast lhsT)
PCOLS = 681

_BUILD_CACHE = {}


def _build(n_samples, L):
    """Build the Bass module for one core processing `n_samples` of length L."""
    from contextlib import ExitStack
    import concourse.bass as bass  # noqa: F401
    import concourse.mybir as mybir
    import concourse.tile as tile
    from concourse import bacc

    dt = mybir.dt
    F32 = dt.float32
    BF16 = dt.bfloat16
    AF = mybir.ActivationFunctionType
    OP = mybir.AluOpType
    AX = mybir.AxisListType

    NL = L // 128      # 128-col l-tiles per sample (n index, l = 32*p + n*... )
    NJ = NL // 4       # transpose groups (4 tiles of 128 -> one 512-col copy)
    NG = L // 512      # conv l-groups
    NCH = 4            # x-load chunks per sample
    NLC = NL // NCH    # n-slots per chunk
    NROW = L // P      # rows of consecutive l per partition (=32 at L=4096)
    assert NL % NCH == 0 and NL % 4 == 0 and NG >= 2

    nc = bacc.Bacc(None, target_bir_lowering=False)
    names = {}

    with tile.TileContext(nc) as tc, ExitStack() as ctx:
        dram = ctx.enter_context(tc.tile_pool(name="dram", bufs=1, space="DRAM"))
        x_in = dram.tile([n_samples, L, C], F32, kind="ExternalInput")
        bank_d = dram.tile([P, KN, 2, KK, O], BF16, kind="ExternalInput")
        par_d = dram.tile([P, PCOLS], F32, kind="ExternalInput")
        con_d = dram.tile([P, 256], BF16, kind="ExternalInput")
        out_d = dram.tile([n_samples, O, L], F32, kind="ExternalOutput")
        names["x"] = x_in.name
        names["bank"] = bank_d.name
        names["params"] = par_d.name
        names["consts"] = con_d.name
        names["out"] = out_d.name

        cpool = ctx.enter_context(tc.tile_pool(name="const", bufs=1))
        xfp = ctx.enter_context(tc.tile_pool(name="xf", bufs=2))
        xnp = ctx.enter_context(tc.tile_pool(name="xnat", bufs=2 * NCH))
        xtp = ctx.enter_context(tc.tile_pool(name="xt", bufs=2))
        tpsum = ctx.enter_context(tc.tile_pool(name="tpsum", bufs=3, space="PSUM"))
        cpsum = ctx.enter_context(tc.tile_pool(name="cpsum", bufs=4, space="PSUM"))
        hpsum = ctx.enter_context(tc.tile_pool(name="hpsum", bufs=1, space="PSUM"))
        hsb = ctx.enter_context(tc.tile_pool(name="hsb", bufs=2))
        accp = ctx.enter_context(tc.tile_pool(name="accp", bufs=2))
        aggp = ctx.enter_context(tc.tile_pool(name="aggp", bufs=2))
        outp = ctx.enter_context(tc.tile_pool(name="outp", bufs=3))

        # constants: consts first (identity unblocks transposes), bank later
        con = cpool.tile([P, 256], BF16)
        nc.sync.dma_start(out=con[:], in_=con_d[:])
        par = cpool.tile([P, PCOLS], F32)
        nc.sync.dma_start(out=par[:], in_=par_d[:])

        ident = con[:, 0:128]
        ones_bf = con[0:1, 128:256]
        ones_f32 = par[0:1, _PC_ONES:_PC_ONES + 128]
        bn_scale = par[0:16, _PC_BNS:_PC_BNS + 1]
        bn_bias = par[0:16, _PC_BNB:_PC_BNB + 1]
        catT = par[0:17, _PC_CAT:_PC_CAT + O + KK + KN]

        # h17: attention hidden vector with a constant-1 17th row (bias fold)
        h17 = cpool.tile([17, 1], F32)
        nc.vector.memset(h17[:], 1.0)

        xn_chunks = {}   # b -> list of NCH bf16 tiles [P, NLC, C]
        xt_tiles = {}
        pp_tiles = {}
        agg_tiles = {}

        def load(b):
            """Issue x[b] chunk loads. Sample 0 goes HWDGE f32 + DVE cast
            (fast start, separate queue); the rest use SWDGE casting DMA.
            Layout: partition p holds rows l in [NROW*p, NROW*(p+1))."""
            xv = x_in[b].rearrange("(p n) c -> p n c", p=P)
            tiles = []
            for ch in range(NCH):
                sl = slice(ch * NLC, (ch + 1) * NLC)
                xnt = xnp.tile([P, NLC, C], BF16, tag="xn")
                if b == 0:
                    xf = xfp.tile([P, NLC, C], F32, tag="xf")
                    nc.sync.dma_start(out=xf[:], in_=xv[:, sl, :])
                    nc.vector.tensor_copy(xnt[:], xf[:])
                else:
                    nc.gpsimd.dma_start(out=xnt[:], in_=xv[:, sl, :])
                tiles.append(xnt)
            xn_chunks[b] = tiles

        def tp_group(b, j):
            """Transpose 4 n-slots (all l = NROW*p + n) of sample b; scatter
            into xt with stride NROW to restore l order; pooled row-sums via
            accum. Alternates ACT/DVE."""
            xt = xt_tiles[b]
            pp = pp_tiles[b]
            ci, jj = divmod(j, NJ)
            tp = tpsum.tile([P, 512], BF16, tag="tp")
            for q in range(4):
                idx = jj * 4 + q
                chunk = xn_chunks[b][idx // NLC]
                nc.tensor.transpose(
                    tp[:, q * 128:(q + 1) * 128],
                    chunk[:, idx % NLC, ci * 128:(ci + 1) * 128],
                    ident,
                )
            # dst: l = NROW*l2 + (4*jj + q);  src tp col = q*128 + l2
            xtv = xt[:, ci, 1:1 + L].rearrange("p (l2 n) -> p l2 n", n=NROW)
            dst = xtv[:, :, 4 * jj:4 * jj + 4]
            src = tp.rearrange("p (q l2) -> p l2 q", q=4)
            acc = pp[:, ci, jj:jj + 1]
            if j % 2 == 0:
                nc.scalar.activation(dst, src, AF.Copy, accum_out=acc)
            else:
                nc.vector.tensor_scalar(dst, src, 0.0, 0.0, op0=OP.add,
                                        op1=OP.add, accum_out=acc)

        def tp_begin(b):
            xt = xtp.tile([P, 2, L + 2], BF16, tag="xt")
            nc.vector.memset(xt[:, :, 0:1], 0.0)
            nc.vector.memset(xt[:, :, L + 1:L + 2], 0.0)
            xt_tiles[b] = xt
            pp_tiles[b] = hsb.tile([P, 2, NJ], F32, tag="pp", name="pp")

        def head(b):
            """Attention head for sample b (small serial chain)."""
            pp = pp_tiles[b]
            pooledT = hsb.tile([P, 2], F32, tag="pooledT")
            nc.vector.tensor_reduce(pooledT[:], pp[:], AX.X, op=OP.add)

            h_ps = hpsum.tile([16, 1], F32, tag="h")
            for ci in range(2):
                nc.tensor.matmul(
                    h_ps[:],
                    par[:, ci * 16:(ci + 1) * 16],
                    pooledT[:, ci:ci + 1],
                    start=(ci == 0),
                    stop=(ci == 1),
                )
            nc.scalar.activation(h17[0:16, :], h_ps[:], AF.Relu,
                                 bias=bn_bias, scale=bn_scale)

            z_ps = hpsum.tile([1, O + KK + KN], F32, tag="h")
            nc.tensor.matmul(z_ps[:], h17[:], catT, start=True, stop=True)
            att = hsb.tile([1, O + KK + KN], F32, tag="att")
            nc.scalar.activation(att[0:1, 0:O + KK], z_ps[0:1, 0:O + KK],
                                 AF.Sigmoid)
            ksum = hsb.tile([1, 2], F32, tag="ksum")
            nc.scalar.activation(
                att[0:1, O + KK:O + KK + KN],
                z_ps[0:1, O + KK:O + KK + KN],
                AF.Exp,
                accum_out=ksum[0:1, 0:1],
            )
            nc.vector.reciprocal(ksum[0:1, 1:2], ksum[0:1, 0:1])

            chz_ps = hpsum.tile([P, 2], F32, tag="h")
            for ci in range(2):
                nc.tensor.matmul(
                    chz_ps[:, ci:ci + 1],
                    par[0:17, _PC_CHT + ci * 128:_PC_CHT + (ci + 1) * 128],
                    h17[:],
                    start=True,
                    stop=True,
                )
            ch_sb = hsb.tile([P, 2], F32, tag="ch_sb")
            for ci in range(2):
                nc.scalar.activation(ch_sb[:, ci:ci + 1], chz_ps[:, ci:ci + 1],
                                     AF.Sigmoid)

            # fo[k, o] = sp_att[k] * fil_att[o] / ksum, broadcast to 128 parts
            fo = hsb.tile([1, KK, O], BF16, tag="fo")
            for k in range(KK):
                nc.vector.tensor_scalar(
                    fo[0:1, k],
                    att[0:1, 0:O],
                    att[0:1, O + k:O + k + 1],
                    ksum[0:1, 1:2],
                    op0=OP.mult,
                    op1=OP.mult,
                )
            fo_bc = hsb.tile([P, KK, O], BF16, tag="fo_bc")
            fo_flat = fo.rearrange("p a b -> p (a b)")
            fobc_flat = fo_bc.rearrange("p a b -> p (a b)")
            for s0, s1 in ((0, 512), (512, KK * O)):
                fo_ps = hpsum.tile([P, 512], F32, tag="h")
                nc.tensor.matmul(fo_ps[:, 0:s1 - s0], ones_bf,
                                 fo_flat[0:1, s0:s1], start=True, stop=True)
                nc.vector.tensor_copy(fobc_flat[:, s0:s1], fo_ps[:, 0:s1 - s0])

            e_ps = hpsum.tile([P, KN], F32, tag="h")
            nc.tensor.matmul(e_ps[:], ones_f32, att[0:1, O + KK:O + KK + KN],
                             start=True, stop=True)
            kerb = hsb.tile([P, KN], F32, tag="kerb")
            nc.vector.tensor_copy(kerb[:], e_ps[:])
            return ch_sb, fo_bc, kerb

        def agg_half(b, mh, ch_sb, fo_bc, kerb):
            """Aggregate the o-half `mh` of the dynamic weight (DVE)."""
            if mh == 0:
                agg_tiles[b] = aggp.tile([P, 2, KK, O], BF16, tag="aggT",
                                         name="aggT")
                # two ping-pong accumulators, persistent across halves
                agg_tiles[(b, "a0")] = accp.tile([P, 2, KK, O], BF16, tag="a0",
                                                 name="a0")
                agg_tiles[(b, "a1")] = accp.tile([P, 2, KK, O], BF16, tag="a1",
                                                 name="a1")
            aggT = agg_tiles[b]
            sl = slice(mh * 128, (mh + 1) * 128)
            a0 = agg_tiles[(b, "a0")][:, :, :, sl]
            a1 = agg_tiles[(b, "a1")][:, :, :, sl]
            bk = bank[:, :, :, :, sl]
            nc.vector.tensor_scalar_mul(a0, bk[:, 0], kerb[:, 0:1])
            nc.vector.scalar_tensor_tensor(
                a1, bk[:, 1], kerb[:, 1:2], a0, op0=OP.mult, op1=OP.add)
            nc.vector.scalar_tensor_tensor(
                a0, bk[:, 2], kerb[:, 2:3], a1, op0=OP.mult, op1=OP.add)
            nc.vector.scalar_tensor_tensor(
                a1, bk[:, 3], kerb[:, 3:4], a0, op0=OP.mult, op1=OP.add)
            for ci in range(2):
                nc.vector.scalar_tensor_tensor(
                    aggT[:, ci, :, sl],
                    a1[:, ci],
                    ch_sb[:, ci:ci + 1],
                    fo_bc[:, :, sl],
                    op0=OP.mult,
                    op1=OP.mult,
                )

        # ---------------- pipelined schedule ----------------
        load(0)
        if n_samples > 1:
            load(1)
        # bank load after x[0] is queued (HWDGE; needed only for agg at ~20us)
        bank = cpool.tile([P, KN, 2, KK, O], BF16)
        nc.sync.dma_start(out=bank[:], in_=bank_d[:])

        tp_begin(0)
        for j in range(2 * NJ):
            tp_group(0, j)
        hout = head(0)
        agg_half(0, 0, *hout)
        agg_half(0, 1, *hout)

        for b in range(n_samples):
            if b + 2 < n_samples:
                load(b + 2)
            if b + 1 < n_samples:
                tp_begin(b + 1)
            aggT = agg_tiles[b]
            xt = xt_tiles[b]
            g = 0
            for m in range(2):
                osb = outp.tile([P, L], F32, tag="osb")
                for lg in range(NG):
                    cp = cpsum.tile([P, 512], F32, tag="cp")
                    idx = 0
                    for ci in range(2):
                        for k in range(KK):
                            nc.tensor.matmul(
                                cp[:],
                                aggT[:, ci, k, m * 128:(m + 1) * 128],
                                xt[:, ci, lg * 512 + k:lg * 512 + k + 512],
                                start=(idx == 0),
                                stop=(idx == 5),
                            )
                            idx += 1
                    dst = osb[:, lg * 512:(lg + 1) * 512]
                    if lg % 2 == 0:
                        nc.vector.tensor_copy(dst, cp[:])
                    else:
                        nc.scalar.activation(dst, cp[:], AF.Copy)
                    # interleave next sample's transposes / head into the conv
                    if b + 1 < n_samples:
                        if g < NG:
                            tp_group(b + 1, 2 * g)
                            tp_group(b + 1, 2 * g + 1)
                        if g == min(NG, 2 * NG - 3):
                            hout = head(b + 1)
                        elif g == min(NG + 2, 2 * NG - 2):
                            agg_half(b + 1, 0, *hout)
                        elif g == min(NG + 4, 2 * NG - 1):
                            agg_half(b + 1, 1, *hout)
                    # quarter-granularity output stores
                    if lg % 2 == 1:
                        h0 = (lg - 1) * 512
                        nc.sync.dma_start(
                            out=out_d[b, m * 128:(m + 1) * 128, h0:h0 + 1024],
                            in_=osb[:, h0:h0 + 1024],
                        )
                    g += 1

    nc.compile()
    return nc, names


def _host_prep(weight, w_fc, bn_gamma, bn_beta, bn_mean, bn_var,
               w_ch, b_ch, w_fil, b_fil, w_sp, b_sp, w_ker, b_ker, L):
    bf16 = ml_dtypes.bfloat16
    bank = np.ascontiguousarray(
        weight.reshape(KN, O, 2, P, KK).transpose(3, 0, 2, 4, 1)
    ).astype(bf16)  # [p, n, ci, k, o]

    par = np.zeros((P, PCOLS), np.float32)
    # w_fcT [p, ci*16+a]
    par[:, 0:32] = w_fc.T.reshape(2, P, A).transpose(1, 0, 2).reshape(P, 32)
    par[0:A, _PC_CHT:_PC_CHT + C] = w_ch.T
    par[16, _PC_CHT:_PC_CHT + C] = b_ch
    par[0:A, _PC_CAT:_PC_CAT + O] = w_fil.T
    par[0:A, _PC_CAT + O:_PC_CAT + O + KK] = w_sp.T
    par[0:A, _PC_CAT + O + KK:_PC_CAT + O + KK + KN] = w_ker.T
    par[16, _PC_CAT:_PC_CAT + O] = b_fil
    par[16, _PC_CAT + O:_PC_CAT + O + KK] = b_sp
    par[16, _PC_CAT + O + KK:_PC_CAT + O + KK + KN] = b_ker
    r = bn_gamma / np.sqrt(bn_var + BN_EPS)
    par[0:A, _PC_BNS] = r / L
    par[0:A, _PC_BNB] = bn_beta - bn_mean * r
    par[0, _PC_ONES:_PC_ONES + 128] = 1.0

    con = np.zeros((P, 256), bf16)
    con[:, 0:128] = np.eye(P, dtype=bf16)
    con[0, 128:256] = bf16(1.0)
    return bank, par, con


def run_cores(x, weight, w_fc, bn_gamma, bn_beta, bn_mean, bn_var,
              w_ch, b_ch, w_fil, b_fil, w_sp, b_sp, w_ker, b_ker,
              n_cores=N_CORES, trace=False):
    """Shard x over cores, run, gather. x: [n_cores*ns, L, C] float32."""
    from concourse.bass_utils import run_bass_kernel_spmd

    x = np.ascontiguousarray(np.asarray(x, np.float32))
    ntot, L, _ = x.shape
    assert ntot % n_cores == 0
    ns = ntot // n_cores

    key = (ns, L)
    if key not in _BUILD_CACHE:
        _BUILD_CACHE[key] = _build(ns, L)
    nc, names = _BUILD_CACHE[key]

    bank, par, con = _host_prep(
        np.asarray(weight, np.float32), np.asarray(w_fc, np.float32),
        np.asarray(bn_gamma, np.float32), np.asarray(bn_beta, np.float32),
        np.asarray(bn_mean, np.float32), np.asarray(bn_var, np.float32),
        np.asarray(w_ch, np.float32), np.asarray(b_ch, np.float32),
        np.asarray(w_fil, np.float32), np.asarray(b_fil, np.float32),
        np.asarray(w_sp, np.float32), np.asarray(b_sp, np.float32),
        np.asarray(w_ker, np.float32), np.asarray(b_ker, np.float32), L)

    in_maps = []
    for c in range(n_cores):
        in_maps.append({
            names["x"]: np.ascontiguousarray(x[c * ns:(c + 1) * ns]),
            names["bank"]: bank,
            names["params"]: par,
            names["consts"]: con,
        })
    res = run_bass_kernel_spmd(nc, in_maps, core_ids=list(range(n_cores)),
                               trace=trace)
    out = np.concatenate([r[names["out"]] for r in res.results], axis=0)
    return out, res


def kernel(**inputs):
    out, _ = run_cores(**inputs)
    return out
